# revision 8
# baseline (speedup 1.0000x reference)
"""Trainium2 Bass kernel for nn_CrossEntropyLoss_2585570312585.

Reference computation (jax):
    cw = where(cw == 0, cw[0], cw)                      # [5]
    gold2dim   = argmax(gold, axis=class)               # [256,384]
    prediction = argmax(pred, axis=class)
    pred_fp    = where(gold2dim > 0, 0,
                       where(prediction == gold2dim, 0, prediction))
    weight_fp  = cw[pred_fp]
    loss = -(weight + weight_fp) * sum_c(gold * log(pred + 1e-8))
    out  = mean(loss)                                   # scalar

Algebraic restructuring used here (exactly equivalent up to fp assoc):
  * pred_fp = where(gold2dim > 0, 0, prediction)  -- the inner where is a
    no-op when gold2dim == 0 since prediction == gold2dim implies
    prediction == 0 there.
  * gold2dim > 0  <=>  max(g[1:5]) > g[0]   (exact, incl. argmax ties)
  * cw[prediction] = sum_c cw_c * (p_c == max_c p_c)  (exact except exact
    float ties between classes, which double-count; measure-zero inputs)
  * The scalar mean decomposes into per-class partial sums, so the device
    returns per-partition partials and the host applies cw and the final
    tiny reduction during the gather step.

Sharding: the 256x384 = 98304-pixel plane is split into 8 contiguous
chunks of 12288 pixels (one per NeuronCore), laid out as [128 partitions
x 96 pixels], with the 5 classes packed side by side in the free dim
([128, 480] tiles). The host pre-packs per-core contiguous buffers so
each input is a single dense DMA.

Device per core:
  L    = log(pred + 1e-8)                      (ACT)
  prod = gold * L                              (GpSimd)
  u    = sum_c prod                            (DVE reduce, class axis)
  m    = max_c pred                            (DVE reduce, class axis)
  eq   = (pred == m)                           (DVE, [128,5,96])
  gr   = max(g1..g4)                           (GpSimd, 2 ops)
  gmask= gr > g0                               (DVE)
  vu   = (gmask - 1) * u                       (DVE fused)
  z    = eq * vu                               (DVE, broadcast)
  accz = sum_pixels z          -> [128, 5]     (DVE reduce)
  base = gmask * cw0 + weight                  (DVE fused)
  acc1 = sum_pixels base * u   -> [128, 1]     (DVE fused reduce)
Host: loss = -(sum acc1 - sum_c cw_c * sum accz_c) / 98304
"""

import os
import sys

import numpy as np


def _ensure_concourse():
    try:
        import concourse  # noqa: F401
        return
    except ImportError:
        pass
    for p in ("/opt/trn_rl_repo", "/root/.axon_site/_ro/trn_rl_repo"):
        if os.path.isdir(p) and p not in sys.path:
            sys.path.insert(0, p)
    import concourse  # noqa: F401


_ensure_concourse()

import concourse.bass as bass  # noqa: E402
import concourse.tile as tile  # noqa: E402
from concourse import bacc, mybir  # noqa: E402
from concourse.bass_utils import run_bass_kernel_spmd  # noqa: E402

N_CORES = 8
H, W = 256, 384
N_PIX = H * W                      # 98304
PIX_PER_CORE = N_PIX // N_CORES    # 12288
P = 128                            # partitions
F = PIX_PER_CORE // P              # 96 free-dim pixels per partition
C = 5                              # classes
EPS = 1e-8

F32 = mybir.dt.float32
Alu = mybir.AluOpType
ActFn = mybir.ActivationFunctionType

# Set by callers that want a profile; results stashed in LAST_RESULTS.
TRACE = False
LAST_RESULTS = None
PROD_ON_POOL = True

_PROGRAM_CACHE = {}


def _build_program(cw0: float):
    """Build + compile the per-core Bass program (same program on all 8
    cores; only the data differs). cw0 is baked as an immediate."""
    nc = bacc.Bacc(
        "TRN2",
        target_bir_lowering=False,
        debug=False,
        enable_asserts=False,
        num_devices=N_CORES,
    )

    pred_d = nc.dram_tensor("pred", [P, C * F], F32, kind="ExternalInput").ap()
    gold_d = nc.dram_tensor("gold", [P, C * F], F32, kind="ExternalInput").ap()
    wgt_d = nc.dram_tensor("wgt", [P, F], F32, kind="ExternalInput").ap()
    acc_d = nc.dram_tensor("acc", [P, 6], F32, kind="ExternalOutput").ap()

    with tile.TileContext(nc) as tc:
        with tc.tile_pool(name="main", bufs=1) as pool:
            # eps bias tile for ln(p + eps)
            eps_t = pool.tile([P, 1], F32)
            nc.vector.memset(eps_t[:], EPS)

            # Warm up the ACT ln table before the input DMAs land.
            warm = pool.tile([P, 1], F32)
            nc.vector.memset(warm[:], 1.0)
            nc.scalar.activation(warm[:], warm[:], ActFn.Ln, bias=eps_t[:])

            p_t = pool.tile([P, C * F], F32)
            nc.sync.dma_start(out=p_t[:], in_=pred_d)
            g_t = pool.tile([P, C * F], F32)
            nc.sync.dma_start(out=g_t[:], in_=gold_d)
            w_t = pool.tile([P, F], F32)
            nc.sync.dma_start(out=w_t[:], in_=wgt_d)

            # class-major views
            p_cj = p_t[:].rearrange("p (c j) -> p c j", c=C)   # [128,5,96]
            p_jc = p_t[:].rearrange("p (c j) -> p j c", c=C)   # [128,96,5]
            g_cj = g_t[:].rearrange("p (c j) -> p c j", c=C)

            # L = ln(pred + eps)
            L_t = pool.tile([P, C * F], F32)
            nc.scalar.activation(L_t[:], p_t[:], ActFn.Ln, bias=eps_t[:])

            # prod = gold * L
            prod_t = pool.tile([P, C * F], F32)
            prod_eng = nc.gpsimd if PROD_ON_POOL else nc.vector
            prod_eng.tensor_tensor(prod_t[:], g_t[:], L_t[:], op=Alu.mult)

            # u = sum_c prod   [128,96]
            u_t = pool.tile([P, F], F32)
            prod_jc = prod_t[:].rearrange("p (c j) -> p j c", c=C)
            nc.vector.tensor_reduce(
                u_t[:], prod_jc, axis=mybir.AxisListType.X, op=Alu.add
            )

            # m = max_c pred   [128,96]
            m_t = pool.tile([P, F], F32)
            nc.vector.tensor_reduce(
                m_t[:], p_jc, axis=mybir.AxisListType.X, op=Alu.max
            )

            # eq = (pred == m) [128,5,96]
            eq_t = pool.tile([P, C * F], F32)
            eq_cj = eq_t[:].rearrange("p (c j) -> p c j", c=C)
            m_b = m_t[:].unsqueeze(1).broadcast_to([P, C, F])
            nc.vector.tensor_tensor(eq_cj, p_cj, m_b, op=Alu.is_equal)

            # gr = max(g1..g4)  (strided class reduce; Pool can't do max)
            g_jc = g_t[:].rearrange("p (c j) -> p j c", c=C)
            gr_t = pool.tile([P, F], F32)
            nc.vector.tensor_reduce(
                gr_t[:], g_jc[:, :, 1:5], axis=mybir.AxisListType.X, op=Alu.max
            )

            # gmask = gr > g0
            gmask_t = pool.tile([P, F], F32)
            nc.vector.tensor_tensor(
                gmask_t[:], gr_t[:], g_t[:, 0:F], op=Alu.is_gt
            )

            # vu = (gmask - 1) * u
            vu_t = pool.tile([P, F], F32)
            nc.vector.scalar_tensor_tensor(
                vu_t[:], gmask_t[:], 1.0, u_t[:],
                op0=Alu.subtract, op1=Alu.mult,
            )

            # z = eq * vu  [128,5,96]
            z_t = pool.tile([P, C * F], F32)
            z_cj = z_t[:].rearrange("p (c j) -> p c j", c=C)
            vu_b = vu_t[:].unsqueeze(1).broadcast_to([P, C, F])
            nc.vector.tensor_tensor(z_cj, eq_cj, vu_b, op=Alu.mult)

            # accumulator tile: col0 = acc1, cols 1..5 = accz
            acc_t = pool.tile([P, 6], F32)
            nc.vector.tensor_reduce(
                acc_t[:, 1:6], z_cj, axis=mybir.AxisListType.X, op=Alu.add
            )

            # base = gmask * cw0 + w
            base_t = pool.tile([P, F], F32)
            nc.vector.scalar_tensor_tensor(
                base_t[:], gmask_t[:], float(cw0), w_t[:],
                op0=Alu.mult, op1=Alu.add,
            )

            # acc1 = sum_pixels base * u  (TTR breaks on this HW path; use
            # mult + reduce)
            bu_t = pool.tile([P, F], F32)
            nc.vector.tensor_tensor(bu_t[:], base_t[:], u_t[:], op=Alu.mult)
            nc.vector.tensor_reduce(
                acc_t[:, 0:1], bu_t[:], axis=mybir.AxisListType.X, op=Alu.add
            )

            nc.sync.dma_start(out=acc_d, in_=acc_t[:])

    nc.compile()
    return nc


def _pack_per_core(arr5: np.ndarray, core: int) -> np.ndarray:
    """arr5: [5, 98304] -> per-core [128, 480] contiguous, class-major in
    the free dim (partition p holds pixels [p*96, (p+1)*96) of the core's
    chunk for each class)."""
    chunk = arr5[:, core * PIX_PER_CORE : (core + 1) * PIX_PER_CORE]
    return np.ascontiguousarray(
        chunk.reshape(C, P, F).transpose(1, 0, 2).reshape(P, C * F)
    )


def kernel(pred, gold, weight, clss_weight_list):
    global LAST_RESULTS

    pred = np.asarray(pred, dtype=np.float32)
    gold = np.asarray(gold, dtype=np.float32)
    weight = np.asarray(weight, dtype=np.float32)
    cw = np.asarray(clss_weight_list, dtype=np.float32)[0]  # [5]
    cw_adj = np.where(cw == 0, cw[0], cw).astype(np.float32)
    cw0 = float(cw_adj[0])

    key = np.float32(cw0).tobytes()
    nc = _PROGRAM_CACHE.get(key)
    if nc is None:
        nc = _build_program(cw0)
        _PROGRAM_CACHE[key] = nc

    p5 = pred[0].reshape(C, N_PIX)
    g5 = gold[0].reshape(C, N_PIX)
    w1 = weight[0].reshape(N_PIX)

    in_maps = []
    for k in range(N_CORES):
        in_maps.append(
            {
                "pred": _pack_per_core(p5, k),
                "gold": _pack_per_core(g5, k),
                "wgt": np.ascontiguousarray(
                    w1[k * PIX_PER_CORE : (k + 1) * PIX_PER_CORE].reshape(P, F)
                ),
            }
        )

    res = run_bass_kernel_spmd(
        nc, in_maps, list(range(N_CORES)), trace=TRACE
    )
    LAST_RESULTS = res

    total = 0.0
    cw64 = cw_adj.astype(np.float64)
    for k in range(N_CORES):
        acc = np.asarray(res.results[k]["acc"], dtype=np.float64)  # [128,6]
        total += acc[:, 0].sum()
        total -= (cw64 * acc[:, 1:6].sum(axis=0)).sum()

    loss = -total / N_PIX
    return np.float32(loss)


# revision 9
# speedup vs baseline: 1.1690x; 1.1690x over previous
"""Trainium2 Bass kernel for nn_CrossEntropyLoss_2585570312585.

Reference computation (jax):
    cw = where(cw == 0, cw[0], cw)                      # [5]
    gold2dim   = argmax(gold, axis=class)               # [256,384]
    prediction = argmax(pred, axis=class)
    pred_fp    = where(gold2dim > 0, 0,
                       where(prediction == gold2dim, 0, prediction))
    weight_fp  = cw[pred_fp]
    loss = -(weight + weight_fp) * sum_c(gold * log(pred + 1e-8))
    out  = mean(loss)                                   # scalar

Algebraic restructuring (exactly equivalent up to fp assoc):
  * pred_fp = where(gold2dim > 0, 0, prediction)  -- the inner where is a
    no-op when gold2dim == 0 since prediction == gold2dim implies
    prediction == 0 there.
  * gold2dim > 0  <=>  max(g[1:5]) > g[0]   (exact, incl. argmax ties)
  * cw[prediction] = sum_c cw_c * (p_c == max_c p_c)  (exact except exact
    float ties between classes, which double-count; measure-zero inputs)
  * The scalar mean decomposes into per-class partial sums, so the device
    returns per-partition partials and the host applies cw and the final
    tiny reduction during the gather step.

Sharding: the 256x384 = 98304-pixel plane is split into 8 contiguous
chunks of 12288 pixels (one per NeuronCore), laid out as [128 partitions
x 96 pixels]. The host pre-packs per-core buffers CLASS-MINOR
(interleaved: free index j*5 + c) so every class reduction on DVE is
inner-contiguous (~645 ns vs ~950 ns for strided). gold and weight are
packed into one buffer so each core does two input DMAs total, issued
from different DGEs (SP HWDGE + Pool SWDGE) for parallel descriptor gen.

HW-measured notes driving the design (see session notes):
  * GpSimd elementwise compute contends with DVE on SBUF ports (measured
    2.5x slowdown of concurrent DVE ops) -> all compute on DVE, ACT does
    ln + casts, Pool only issues a DMA.
  * tensor_tensor_reduce / DMA accum / Pool max are rejected or broken on
    this toolchain -> plain mult+reduce forms only.
  * bf16 tensor_tensor gets 2x (400 ns vs 648 ns at [128,480]); used for
    the prod and z products where rounding provably cannot bias the
    result beyond ~1e-5 relative.

Device per core (all tiles [128, 480] interleaved unless noted):
  L    = ln(pred + 1e-8)  -> bf16              (ACT)
  gb   = bf16(gold)                            (ACT copy)
  prod = gb * L           (bf16 2x)            (DVE)
  u    = sum_c prod        -> [128,96] f32     (DVE reduce, contiguous)
  m    = max_c pred        -> [128,96] f32     (DVE reduce, contiguous)
  eq   = (pred == m_bcast) -> bf16             (DVE)
  gr   = max(g1..g4)       -> [128,96]         (DVE reduce, contiguous)
  gmask= gr > g0                               (DVE, g0 stride-5 view)
  vu   = (gmask - 1) * u   -> bf16             (DVE fused stt)
  z    = eq * vu_bcast     (bf16 2x)           (DVE)
  accz = sum_pixels z      -> [128, 5] f32     (DVE reduce, strided)
  base = gmask * cw0 + weight                  (DVE fused stt)
  bu   = base * u ; acc1 = sum_pixels bu       (DVE)
Host: loss = -(sum acc1 - sum_c cw_c * sum accz_c) / 98304
"""

import os
import sys

import numpy as np


def _ensure_concourse():
    try:
        import concourse  # noqa: F401
        return
    except ImportError:
        pass
    for p in ("/opt/trn_rl_repo", "/root/.axon_site/_ro/trn_rl_repo"):
        if os.path.isdir(p) and p not in sys.path:
            sys.path.insert(0, p)
    import concourse  # noqa: F401


_ensure_concourse()

import concourse.bass as bass  # noqa: E402
import concourse.tile as tile  # noqa: E402
from concourse import bacc, mybir  # noqa: E402
from concourse.bass_utils import run_bass_kernel_spmd  # noqa: E402

N_CORES = 8
H, W = 256, 384
N_PIX = H * W                      # 98304
PIX_PER_CORE = N_PIX // N_CORES    # 12288
P = 128                            # partitions
F = PIX_PER_CORE // P              # 96 free-dim pixels per partition
C = 5                              # classes
EPS = 1e-8

F32 = mybir.dt.float32
BF16 = mybir.dt.bfloat16
Alu = mybir.AluOpType
ActFn = mybir.ActivationFunctionType
AxX = mybir.AxisListType.X

# Set by callers that want a profile; results stashed in LAST_RESULTS.
TRACE = False
LAST_RESULTS = None

_PROGRAM_CACHE = {}


def _build_program(cw0: float):
    """Build + compile the per-core Bass program (same program on all 8
    cores; only the data differs). cw0 is baked as an immediate."""
    nc = bacc.Bacc(
        "TRN2",
        target_bir_lowering=False,
        debug=False,
        enable_asserts=False,
        num_devices=N_CORES,
    )

    # pred: [128, 480] interleaved (j*5 + c); goldw: gold interleaved 480
    # cols then weight 96 cols.
    pred_d = nc.dram_tensor("pred", [P, C * F], F32, kind="ExternalInput").ap()
    goldw_d = nc.dram_tensor(
        "goldw", [P, C * F + F], F32, kind="ExternalInput"
    ).ap()
    acc_d = nc.dram_tensor("acc", [P, 6], F32, kind="ExternalOutput").ap()

    with tile.TileContext(nc) as tc:
        with tc.tile_pool(name="main", bufs=1) as pool:
            # eps bias tile for ln(p + eps)
            eps_t = pool.tile([P, 1], F32)
            nc.vector.memset(eps_t[:], EPS)

            # Warm up the ACT ln table before the input DMAs land.
            warm = pool.tile([P, 1], F32)
            nc.vector.memset(warm[:], 1.0)
            nc.scalar.activation(warm[:], warm[:], ActFn.Ln, bias=eps_t[:])

            p_t = pool.tile([P, C * F], F32)
            nc.sync.dma_start(out=p_t[:], in_=pred_d)
            gw_t = pool.tile([P, C * F + F], F32)
            nc.gpsimd.dma_start(out=gw_t[:], in_=goldw_d)

            # interleaved views: [128, 96(j), 5(c)], inner (class) stride 1
            p_jc = p_t[:].rearrange("p (j c) -> p j c", c=C)
            g_jc = gw_t[:, 0 : C * F].rearrange("p (j c) -> p j c", c=C)
            w_v = gw_t[:, C * F : C * F + F]

            # L = ln(pred + eps), bf16 out
            L_t = pool.tile([P, C * F], BF16)
            nc.scalar.activation(L_t[:], p_t[:], ActFn.Ln, bias=eps_t[:])

            # gb = bf16(gold) on ACT (idle; keeps DVE free)
            gb_t = pool.tile([P, C * F], BF16)
            nc.scalar.copy(gb_t[:], gw_t[:, 0 : C * F])

            # prod = gb * L (bf16 2x)
            prod_t = pool.tile([P, C * F], BF16)
            nc.vector.tensor_tensor(prod_t[:], gb_t[:], L_t[:], op=Alu.mult)

            # u = sum_c prod  [128,96] f32 (inner-contiguous reduce)
            u_t = pool.tile([P, F], F32)
            nc.vector.tensor_reduce(
                u_t[:], prod_t[:].rearrange("p (j c) -> p j c", c=C),
                axis=AxX, op=Alu.add,
            )

            # m = max_c pred  [128,96]
            m_t = pool.tile([P, F], F32)
            nc.vector.tensor_reduce(m_t[:], p_jc, axis=AxX, op=Alu.max)

            # eq = (pred == m) -> bf16, interleaved layout
            eq_t = pool.tile([P, C * F], BF16)
            eq_jc = eq_t[:].rearrange("p (j c) -> p j c", c=C)
            m_b = m_t[:].unsqueeze(2).broadcast_to([P, F, C])
            nc.vector.tensor_tensor(eq_jc, p_jc, m_b, op=Alu.is_equal)

            # gr = max(g1..g4) (inner-contiguous, offset 1)
            gr_t = pool.tile([P, F], F32)
            nc.vector.tensor_reduce(
                gr_t[:], g_jc[:, :, 1:5], axis=AxX, op=Alu.max
            )

            # gmask = gr > g0 (g0 is the stride-5 class-0 view)
            gmask_t = pool.tile([P, F], F32)
            nc.vector.tensor_tensor(
                gmask_t[:], gr_t[:], g_jc[:, :, 0], op=Alu.is_gt
            )

            # vu = (gmask - 1) * u -> bf16
            vu_t = pool.tile([P, F], BF16)
            nc.vector.scalar_tensor_tensor(
                vu_t[:], gmask_t[:], 1.0, u_t[:],
                op0=Alu.subtract, op1=Alu.mult,
            )

            # z = eq * vu (bf16 2x), interleaved
            z_t = pool.tile([P, C * F], BF16)
            z_jc = z_t[:].rearrange("p (j c) -> p j c", c=C)
            vu_b = vu_t[:].unsqueeze(2).broadcast_to([P, F, C])
            nc.vector.tensor_tensor(z_jc, eq_jc, vu_b, op=Alu.mult)

            # accumulator tile: col0 = acc1, cols 1..5 = accz
            acc_t = pool.tile([P, 6], F32)
            # accz_c = sum_j z[j, c]  (strided reduce over j)
            z_cj = z_t[:].rearrange("p (j c) -> p c j", c=C)
            nc.vector.tensor_reduce(acc_t[:, 1:6], z_cj, axis=AxX, op=Alu.add)

            # base = gmask * cw0 + w
            base_t = pool.tile([P, F], F32)
            nc.vector.scalar_tensor_tensor(
                base_t[:], gmask_t[:], float(cw0), w_v,
                op0=Alu.mult, op1=Alu.add,
            )

            # acc1 = sum_pixels base * u
            bu_t = pool.tile([P, F], F32)
            nc.vector.tensor_tensor(bu_t[:], base_t[:], u_t[:], op=Alu.mult)
            nc.vector.tensor_reduce(acc_t[:, 0:1], bu_t[:], axis=AxX, op=Alu.add)

            nc.sync.dma_start(out=acc_d, in_=acc_t[:])

    nc.compile()
    return nc


def _interleave(arr5: np.ndarray, core: int) -> np.ndarray:
    """arr5: [5, 98304] -> per-core [128, 480] class-minor (free index
    j*5 + c)."""
    chunk = arr5[:, core * PIX_PER_CORE : (core + 1) * PIX_PER_CORE]
    # [5, 128, 96] -> [128, 96, 5] -> [128, 480]
    return chunk.reshape(C, P, F).transpose(1, 2, 0).reshape(P, C * F)


def kernel(pred, gold, weight, clss_weight_list):
    global LAST_RESULTS

    pred = np.asarray(pred, dtype=np.float32)
    gold = np.asarray(gold, dtype=np.float32)
    weight = np.asarray(weight, dtype=np.float32)
    cw = np.asarray(clss_weight_list, dtype=np.float32)[0]  # [5]
    cw_adj = np.where(cw == 0, cw[0], cw).astype(np.float32)
    cw0 = float(cw_adj[0])

    key = np.float32(cw0).tobytes()
    nc = _PROGRAM_CACHE.get(key)
    if nc is None:
        nc = _build_program(cw0)
        _PROGRAM_CACHE[key] = nc

    p5 = pred[0].reshape(C, N_PIX)
    g5 = gold[0].reshape(C, N_PIX)
    w1 = weight[0].reshape(N_PIX)

    in_maps = []
    for k in range(N_CORES):
        gw = np.empty((P, C * F + F), dtype=np.float32)
        gw[:, 0 : C * F] = _interleave(g5, k)
        gw[:, C * F :] = w1[k * PIX_PER_CORE : (k + 1) * PIX_PER_CORE].reshape(
            P, F
        )
        in_maps.append(
            {
                "pred": np.ascontiguousarray(_interleave(p5, k)),
                "goldw": gw,
            }
        )

    res = run_bass_kernel_spmd(
        nc, in_maps, list(range(N_CORES)), trace=TRACE
    )
    LAST_RESULTS = res

    total = 0.0
    cw64 = cw_adj.astype(np.float64)
    for k in range(N_CORES):
        acc = np.asarray(res.results[k]["acc"], dtype=np.float64)  # [128,6]
        total += acc[:, 0].sum()
        total -= (cw64 * acc[:, 1:6].sum(axis=0)).sum()

    loss = -total / N_PIX
    return np.float32(loss)
